# revision 33
# baseline (speedup 1.0000x reference)
"""Distributed Trainium2 kernel for AnomalyMoE k-NN retrieval.

reference:  q = l2norm(test[L,N,D]); g = l2norm(normal[L,M,D])
            sim[l,n,m] = q . g ; out = (1 - mean_l max_m sim).reshape(1,1,16,16)

Strategy (8 NeuronCores):
- Shard gallery along M (6400 rows/core).  ALL normalization happens on the
  host: q and g are l2-normalized in f32, scaled by 16 (keeps fp8e4m3
  components out of the subnormal range), cast to fp8.
- The gallery shard is shipped as ONE partition-flat blob [128, 204800]
  laid out per-DMA-tile: each tile (layer, m-range) occupies a contiguous
  column range holding [128p, KC, cols] row-major.  Every gallery DMA is
  then 128 contiguous 2-16KB descriptors -> near-peak HBM bandwidth and
  ~0.25us issue cost (vs 1024 small descriptors at ~300GB/s).  Queries ship
  the same way ([128, L*KC*N] flat).
- Per core the device does ONLY:  dot[n,m] on TensorE fp8 DoubleRow
  pair-matmuls (contraction 256/instruction, FD=512 into f32 PSUM), a
  running elementwise max on DVE (bf16 runmax), one partial 512->1 reduce
  per layer (emitted early, off the critical tail), and for each layer's
  final 256-col tile a direct PSUM reduce + combine.  Warm-up matmuls on
  junk data run during the initial DMA window so HAM is at K=8/8 when the
  real stream starts.
- Gallery DMAs alternate between the two HWDGE rings (sync + scalar);
  tiny per-layer [128,2] result DMAs go out on the GpSimd SWDGE ring.
- Host: cross-core max, /scale^2, mean over layers, 1-x.
"""

import os
import sys
from concurrent.futures import ThreadPoolExecutor

sys.path.insert(0, "/opt/trn_rl_repo")

import numpy as np
import ml_dtypes

import concourse.bacc as bacc
import concourse.mybir as mybir
import concourse.tile as tile
from concourse.bass_utils import run_bass_kernel_spmd

F32 = mybir.dt.float32
BF16 = mybir.dt.bfloat16
ALU = mybir.AluOpType
DR = mybir.MatmulPerfMode.DoubleRow
DT_IN = mybir.dt.float8e4
NP_IN = ml_dtypes.float8_e4m3fn

NCORES = 8
L = 4
D = 1024
N = 256
M_FULL = 51200
MS = M_FULL // NCORES  # 6400 per core
KC = D // 128  # 8 contraction chunks of 128
KP = KC // 2  # 4 DoubleRow pairs
SUPER = 512  # matmul FD (one f32 PSUM bank)
SCALE = 16.0  # host pre-scale on normalized q and g (sims come out x256)
SKEW = int(os.environ.get("KERNEL_SKEW", "1"))  # software-pipeline depth
BUFS_G = int(os.environ.get("KERNEL_BUFS_G", "8"))
BUFS_PM = int(os.environ.get("KERNEL_BUFS_PM", "3"))
N_WARM = int(os.environ.get("KERNEL_WARM", "6"))
KERNEL_TAG = os.environ.get("KERNEL_TAG", "")
NEG = -3.0e38

# per-layer DMA tile column counts.  PE consumes ~263KB/us while the DMA
# supplies ~310KB/us, so early tiles must be SMALL (completion cadence at
# or below the PE cadence) and can ramp up as prefetch slack accumulates.
# Each layer ends with a 256 tile handled by a direct PSUM reduce.
TILES = {
    0: [128, 128, 256, 512, 512, 512, 1024, 1024, 1024, 1024, 256],
    1: [512, 512, 1024, 1024, 1024, 1024, 1024, 256],
    2: [512, 512, 1024, 1024, 1024, 1024, 1024, 256],
    3: [512, 512, 1024, 1024, 1024, 1024, 1024, 256],
}
# supers at columns >= DIRECT_FROM skip the runmax and reduce their PSUM
# directly (tiny tmp + combine): the per-layer partial reduce can then run
# two supers earlier, shrinking the serial DVE chain after the last matmul
DIRECT_FROM = MS - 256 - 1024  # 5120

# work list: (layer, m0, cols, flat column offset in the gallery blob)
GTILES = []
_off = 0
for _lx in range(L):
    _m0 = 0
    for _c in TILES[_lx]:
        GTILES.append((_lx, _m0, _c, _off))
        _m0 += _c
        _off += KC * _c
    assert _m0 == MS
G_FLAT = _off  # 204800


def build():
    nc = bacc.Bacc("TRN2", target_bir_lowering=False, debug=False, num_devices=NCORES)
    g_ext = nc.dram_tensor("g_t", [128, G_FLAT], DT_IN, kind="ExternalInput")
    qt_ext = nc.dram_tensor("q_t", [128, L * KC * N], DT_IN, kind="ExternalInput")
    lmax_ext = nc.dram_tensor("out_lmax", [128, 2 * L], F32, kind="ExternalOutput")

    with tile.TileContext(nc) as tc:
        with (
            tc.tile_pool(name="persist", bufs=1) as pp,
            tc.tile_pool(name="gsup", bufs=BUFS_G) as gpool,
            tc.tile_pool(name="tmpp", bufs=2) as tmppool,
            tc.tile_pool(name="pmp", bufs=BUFS_PM, space="PSUM") as pmpool,
        ):
            # ---- persistent tiles ----
            # qt in TWO tiles (layer 0 / layers 1-3) and lmax per-layer:
            # separate tiles kill the false WAR/RAW deps that a shared tile
            # creates between the layer-0 LDWEIGHTS and the qt-rest DMA, and
            # between a layer's partial reduce and the previous layer's
            # result DMA.
            # qt00 = just layer 0's first k-pair (64KB): the very first
            # LDWEIGHTS waits only on this tiny DMA, not the full qt0 load
            qt00_sb = pp.tile([128, 2, N], DT_IN, name="qt00_sb")
            qt0r_sb = pp.tile([128, KC - 2, N], DT_IN, name="qt0r_sb")
            qtr_sb = pp.tile([128, (L - 1) * KC, N], DT_IN, name="qtr_sb")
            lmax_l = [
                pp.tile([128, 2], F32, name=f"lmax_l{_l}") for _l in range(L)
            ]
            out_all = pp.tile([128, 2 * L], F32, name="out_all")
            runmax = pp.tile([128, 2 * L, SUPER], BF16, name="runmax")
            # runmax columns are initialized by copy the first time each
            # column range is touched (init_w); no memset needed.
            if KERNEL_TAG:
                # cache-buster: changes the BIR so stale NEFF caches miss
                tag_sb = pp.tile([128, 1], F32, name=f"tag_{KERNEL_TAG}")
                nc.gpsimd.memset(tag_sb[:], 1.0)

            # ---- PE warm-up: junk DoubleRow matmuls during the initial DMA
            # window keep the HAM activity monitor busy so the real stream
            # starts at K=8/8 (2.4 GHz) instead of cold (1.2 GHz).
            if N_WARM:
                wq = pp.tile([128, 2, 128], DT_IN, name="warm_q")
                wx = pp.tile([128, 2, SUPER], DT_IN, name="warm_x")
                nc.gpsimd.memset(wq[:], 1.0)
                nc.gpsimd.memset(wx[:], 1.0)
                pmw = pmpool.tile([128, 2, SUPER], F32, name="pm")
                for _ in range(N_WARM):
                    nc.tensor.matmul(
                        pmw[:, 0, :], wq[:], wx[:], start=True, stop=True,
                        perf_mode=DR,
                    )

            def stage_a(lx, m0, cols, off, sidx):
                gsup = gpool.tile([128, KC, cols], DT_IN, name="gsup")
                eng = nc.sync if sidx % 2 == 0 else nc.scalar
                eng.dma_start(
                    gsup[:],
                    g_ext.ap()[:, off : off + KC * cols].rearrange(
                        "p (k m) -> p k m", k=KC
                    ),
                )
                return gsup

            def qt_slice(lx, j, cx):
                if lx == 0:
                    if j == 0:
                        return qt00_sb[:, :, cx * 128 : (cx + 1) * 128]
                    return qt0r_sb[
                        :, 2 * (j - 1) : 2 * j, cx * 128 : (cx + 1) * 128
                    ]
                k0 = (lx - 1) * KC + 2 * j
                return qtr_sb[:, k0 : k0 + 2, cx * 128 : (cx + 1) * 128]

            def stage_b(lx, m0, cols, off, init_w, gsup):
                j0 = 2 * lx
                if m0 == MS - 256:
                    # final 256-col tile of this layer: matmuls, then reduce
                    # the PSUM directly and fold into the (already partially
                    # reduced) lmax -- keeps the post-last-matmul tail to a
                    # PSUM reduce + tiny combine + tiny DMA.
                    pm = pmpool.tile([128, 2, SUPER], F32, name="pm")
                    for cx in range(2):
                        for j in range(KP):
                            nc.tensor.matmul(
                                pm[:, cx, :256],
                                qt_slice(lx, j, cx),
                                gsup[:, 2 * j : 2 * j + 2, :],
                                start=(j == 0),
                                stop=(j == KP - 1),
                                perf_mode=DR,
                            )
                    tmp = tmppool.tile([128, 2], F32, name="tmax")
                    nc.vector.reduce_max(
                        tmp[:], pm[:, :, :256], axis=mybir.AxisListType.X
                    )
                    # combine into the shared output staging tile; ONE DMA
                    # ships all layers at the end
                    nc.vector.tensor_tensor(
                        out=out_all[:, j0 : j0 + 2],
                        in0=lmax_l[lx][:],
                        in1=tmp[:],
                        op=ALU.max,
                    )
                    return
                # consume the gallery tile in 512-col supers
                for s0 in range(0, cols, SUPER):
                    msz = min(SUPER, cols - s0)
                    pm = pmpool.tile([128, 2, SUPER], F32, name="pm")
                    for cx in range(2):
                        for j in range(KP):
                            nc.tensor.matmul(
                                pm[:, cx, :msz],
                                qt_slice(lx, j, cx),
                                gsup[:, 2 * j : 2 * j + 2, s0 : s0 + msz],
                                start=(j == 0),
                                stop=(j == KP - 1),
                                perf_mode=DR,
                            )
                    if m0 + s0 >= DIRECT_FROM:
                        # endgame super: reduce PSUM directly, fold into the
                        # (already partially reduced) lmax
                        tmp = tmppool.tile([128, 2], F32, name="tmax")
                        nc.vector.reduce_max(
                            tmp[:], pm[:, :, :msz], axis=mybir.AxisListType.X
                        )
                        nc.vector.tensor_tensor(
                            out=lmax_l[lx][:],
                            in0=lmax_l[lx][:],
                            in1=tmp[:],
                            op=ALU.max,
                        )
                        continue
                    # running elementwise max into bf16 runmax (DVE); first
                    # touch of a column range copies instead of maxing
                    w = min(init_w, msz)
                    if w > 0:
                        rm = runmax[:, j0 : j0 + 2, :w]
                        nc.vector.tensor_tensor(
                            out=rm, in0=rm, in1=pm[:, :, :w], op=ALU.max
                        )
                    if msz > w:
                        nc.vector.tensor_copy(
                            out=runmax[:, j0 : j0 + 2, w:msz],
                            in_=pm[:, :, w:msz],
                        )
                    init_w = max(init_w, msz)
                if m0 + cols == DIRECT_FROM:
                    # all runmax supers of this layer done: partial reduce
                    # NOW (overlaps the endgame tiles' matmuls instead of
                    # sitting on the serial tail)
                    nc.vector.reduce_max(
                        lmax_l[lx][:],
                        runmax[:, j0 : j0 + 2, :],
                        axis=mybir.AxisListType.X,
                    )

            # init_w = how many runmax columns are valid before this tile
            # (running max of prior super widths within the layer)
            work = []
            _iw = {lx: 0 for lx in range(L)}
            for lx, m0, cols, off in GTILES:
                work.append((lx, m0, cols, off, _iw[lx]))
                _iw[lx] = max(_iw[lx], min(SUPER, cols))
            pending = []
            for sidx, (lx, m0, cols, off, iw) in enumerate(work):
                pending.append((lx, m0, cols, off, iw, stage_a(lx, m0, cols, off, sidx)))
                if sidx == 0:
                    # layer 0's query slices, on the other HWDGE ring so
                    # they drain in parallel with the first gallery tile
                    nc.scalar.dma_start(
                        qt00_sb[:],
                        qt_ext.ap()[:, : 2 * N].rearrange(
                            "p (k n) -> p k n", k=2
                        ),
                    )
                    nc.scalar.dma_start(
                        qt0r_sb[:],
                        qt_ext.ap()[:, 2 * N : KC * N].rearrange(
                            "p (k n) -> p k n", k=KC - 2
                        ),
                    )
                elif sidx == 5:
                    # remaining layers' queries: deferred so they don't
                    # steal early DMA bandwidth from the first gallery tiles
                    nc.scalar.dma_start(
                        qtr_sb[:],
                        qt_ext.ap()[:, KC * N :].rearrange(
                            "p (k n) -> p k n", k=(L - 1) * KC
                        ),
                    )
                if len(pending) > SKEW:
                    stage_b(*pending.pop(0))
            while pending:
                stage_b(*pending.pop(0))
            # single result DMA on the low-latency HWDGE scalar ring
            nc.scalar.dma_start(lmax_ext.ap(), out_all[:])

    nc.compile()
    return nc


_NC_CACHE = None


def _get_nc():
    global _NC_CACHE
    if _NC_CACHE is None:
        _NC_CACHE = build()
    return _NC_CACHE


def _prep_shard(g, c):
    # [L, MS, D] f32 slice -> l2-normalize * SCALE -> fp8 -> per-tile
    # partition-flat blob [128, G_FLAT]
    sl = np.asarray(g[:, c * MS : (c + 1) * MS, :], dtype=np.float32)
    n = np.sqrt(np.einsum("lmd,lmd->lm", sl, sl))
    sl = sl * (SCALE / np.maximum(n, 1e-8))[:, :, None]
    gt = sl.transpose(0, 2, 1).astype(NP_IN)  # [L, D, MS]
    blob = np.empty((128, G_FLAT), dtype=NP_IN)
    for lx, m0, cols, off in GTILES:
        t = gt[lx, :, m0 : m0 + cols].reshape(KC, 128, cols)
        blob[:, off : off + KC * cols] = t.transpose(1, 0, 2).reshape(
            128, KC * cols
        )
    return blob


def _prep_inputs(test_patch_tokens, normal_patch_tokens):
    q = np.asarray(test_patch_tokens, dtype=np.float32)
    g = np.asarray(normal_patch_tokens, dtype=np.float32)
    qn = np.sqrt(np.einsum("lnd,lnd->ln", q, q))
    q = q * (SCALE / np.maximum(qn, 1e-8))[:, :, None]
    qt = q.transpose(0, 2, 1).astype(NP_IN)  # [L, D, N]
    qt_flat = np.ascontiguousarray(
        qt.reshape(L, KC, 128, N).transpose(2, 0, 1, 3).reshape(128, L * KC * N)
    )
    with ThreadPoolExecutor(NCORES) as ex:
        shards = list(ex.map(lambda c: _prep_shard(g, c), range(NCORES)))
    return [{"g_t": shards[c], "q_t": qt_flat} for c in range(NCORES)]


def kernel(test_patch_tokens: np.ndarray, normal_patch_tokens: np.ndarray):
    in_maps = _prep_inputs(test_patch_tokens, normal_patch_tokens)
    nc = _get_nc()
    results = run_bass_kernel_spmd(nc, in_maps, core_ids=list(range(NCORES))).results
    # combine per-shard partials: global max over cores, /SCALE^2, mean over
    # layers, 1-x (tiny: 8*[128,8] values)
    lmax = np.max(
        np.stack([results[c]["out_lmax"] for c in range(NCORES)]), axis=0
    )  # [128, 2L]: column j = layer*2 + chunk
    test_sim = (lmax / (SCALE * SCALE)).reshape(128, L, 2).mean(axis=1)
    out = 1.0 - test_sim.T.reshape(N)  # n = chunk*128 + p
    return out.astype(np.float32).reshape(1, 1, 16, 16)


# revision 34
# speedup vs baseline: 1.0081x; 1.0081x over previous
"""Distributed Trainium2 kernel for AnomalyMoE k-NN retrieval.

reference:  q = l2norm(test[L,N,D]); g = l2norm(normal[L,M,D])
            sim[l,n,m] = q . g ; out = (1 - mean_l max_m sim).reshape(1,1,16,16)

Strategy (8 NeuronCores):
- Shard gallery along M (6400 rows/core).  ALL normalization happens on the
  host: q and g are l2-normalized in f32, scaled by 16 (keeps fp8e4m3
  components out of the subnormal range), cast to fp8.
- The gallery shard is shipped as ONE partition-flat blob [128, 204800]
  laid out per-DMA-tile: each tile (layer, m-range) occupies a contiguous
  column range holding [128p, KC, cols] row-major.  Every gallery DMA is
  then 128 contiguous 2-16KB descriptors -> near-peak HBM bandwidth and
  ~0.25us issue cost (vs 1024 small descriptors at ~300GB/s).  Queries ship
  the same way ([128, L*KC*N] flat).
- Per core the device does ONLY:  dot[n,m] on TensorE fp8 DoubleRow
  pair-matmuls (contraction 256/instruction, FD=512 into f32 PSUM), a
  running elementwise max on DVE (bf16 runmax), one partial 512->1 reduce
  per layer (emitted early, off the critical tail), and for each layer's
  final 256-col tile a direct PSUM reduce + combine.  Warm-up matmuls on
  junk data run during the initial DMA window so HAM is at K=8/8 when the
  real stream starts.
- Gallery DMAs alternate between the two HWDGE rings (sync + scalar);
  tiny per-layer [128,2] result DMAs go out on the GpSimd SWDGE ring.
- Host: cross-core max, /scale^2, mean over layers, 1-x.
"""

import os
import sys
from concurrent.futures import ThreadPoolExecutor

sys.path.insert(0, "/opt/trn_rl_repo")

import numpy as np
import ml_dtypes

import concourse.bacc as bacc
import concourse.mybir as mybir
import concourse.tile as tile
from concourse.bass_utils import run_bass_kernel_spmd

F32 = mybir.dt.float32
BF16 = mybir.dt.bfloat16
ALU = mybir.AluOpType
DR = mybir.MatmulPerfMode.DoubleRow
DT_IN = mybir.dt.float8e4
NP_IN = ml_dtypes.float8_e4m3fn

NCORES = 8
L = 4
D = 1024
N = 256
M_FULL = 51200
MS = M_FULL // NCORES  # 6400 per core
KC = D // 128  # 8 contraction chunks of 128
KP = KC // 2  # 4 DoubleRow pairs
SUPER = 512  # matmul FD (one f32 PSUM bank)
SCALE = 16.0  # host pre-scale on normalized q and g (sims come out x256)
SKEW = int(os.environ.get("KERNEL_SKEW", "1"))  # software-pipeline depth
BUFS_G = int(os.environ.get("KERNEL_BUFS_G", "6"))
BUFS_PM = int(os.environ.get("KERNEL_BUFS_PM", "3"))
N_WARM = int(os.environ.get("KERNEL_WARM", "6"))
KERNEL_TAG = os.environ.get("KERNEL_TAG", "")
NEG = -3.0e38

# per-layer DMA tile column counts.  PE consumes ~263KB/us while the DMA
# supplies ~310KB/us, so early tiles must be SMALL (completion cadence at
# or below the PE cadence) and can ramp up as prefetch slack accumulates.
# Each layer ends with a 256 tile handled by a direct PSUM reduce.
TILES = {
    0: [256, 256, 256, 256, 512, 512, 1024, 1024, 1024, 1024, 256],
    1: [512, 512, 1024, 1024, 1024, 1024, 1024, 256],
    2: [512, 512, 1024, 1024, 1024, 1024, 1024, 256],
    3: [512, 512, 1024, 1024, 1024, 1024, 1024, 256],
}
# supers at columns >= DIRECT_FROM skip the runmax and reduce their PSUM
# directly (tiny tmp + combine): the per-layer partial reduce can then run
# two supers earlier, shrinking the serial DVE chain after the last matmul
DIRECT_FROM = MS - 256 - 1024  # 5120

# work list: (layer, m0, cols, flat column offset in the gallery blob)
GTILES = []
_off = 0
for _lx in range(L):
    _m0 = 0
    for _c in TILES[_lx]:
        GTILES.append((_lx, _m0, _c, _off))
        _m0 += _c
        _off += KC * _c
    assert _m0 == MS
G_FLAT = _off  # 204800


def build():
    nc = bacc.Bacc("TRN2", target_bir_lowering=False, debug=False, num_devices=NCORES)
    g_ext = nc.dram_tensor("g_t", [128, G_FLAT], DT_IN, kind="ExternalInput")
    qt_ext = nc.dram_tensor("q_t", [128, L * KC * N], DT_IN, kind="ExternalInput")
    lmax_ext = nc.dram_tensor("out_lmax", [128, 2 * L], F32, kind="ExternalOutput")

    with tile.TileContext(nc) as tc:
        with (
            tc.tile_pool(name="persist", bufs=1) as pp,
            tc.tile_pool(name="gsup", bufs=BUFS_G) as gpool,
            tc.tile_pool(name="tmpp", bufs=2) as tmppool,
            tc.tile_pool(name="pmp", bufs=BUFS_PM, space="PSUM") as pmpool,
        ):
            # ---- persistent tiles ----
            # qt in TWO tiles (layer 0 / layers 1-3) and lmax per-layer:
            # separate tiles kill the false WAR/RAW deps that a shared tile
            # creates between the layer-0 LDWEIGHTS and the qt-rest DMA, and
            # between a layer's partial reduce and the previous layer's
            # result DMA.
            # qt00 = just layer 0's first k-pair (64KB): the very first
            # LDWEIGHTS waits only on this tiny DMA, not the full qt0 load
            qt00_sb = pp.tile([128, 2, N], DT_IN, name="qt00_sb")
            qt0r_sb = pp.tile([128, KC - 2, N], DT_IN, name="qt0r_sb")
            qtr_sb = pp.tile([128, (L - 1) * KC, N], DT_IN, name="qtr_sb")
            lmax_l = [
                pp.tile([128, 2], F32, name=f"lmax_l{_l}") for _l in range(L)
            ]
            out_all = pp.tile([128, 2 * L], F32, name="out_all")
            runmax = pp.tile([128, 2 * L, SUPER], BF16, name="runmax")
            # runmax columns are initialized by copy the first time each
            # column range is touched (init_w); no memset needed.
            if KERNEL_TAG:
                # cache-buster: changes the BIR so stale NEFF caches miss
                tag_sb = pp.tile([128, 1], F32, name=f"tag_{KERNEL_TAG}")
                nc.gpsimd.memset(tag_sb[:], 1.0)

            # ---- PE warm-up: junk DoubleRow matmuls during the initial DMA
            # window keep the HAM activity monitor busy so the real stream
            # starts at K=8/8 (2.4 GHz) instead of cold (1.2 GHz).
            if N_WARM:
                wq = pp.tile([128, 2, 128], DT_IN, name="warm_q")
                wx = pp.tile([128, 2, SUPER], DT_IN, name="warm_x")
                nc.gpsimd.memset(wq[:], 1.0)
                nc.gpsimd.memset(wx[:], 1.0)
                pmw = pmpool.tile([128, 2, SUPER], F32, name="pm")
                for _ in range(N_WARM):
                    nc.tensor.matmul(
                        pmw[:, 0, :], wq[:], wx[:], start=True, stop=True,
                        perf_mode=DR,
                    )

            def stage_a(lx, m0, cols, off, sidx):
                gsup = gpool.tile([128, KC, cols], DT_IN, name="gsup")
                eng = nc.sync if sidx % 2 == 0 else nc.scalar
                eng.dma_start(
                    gsup[:],
                    g_ext.ap()[:, off : off + KC * cols].rearrange(
                        "p (k m) -> p k m", k=KC
                    ),
                )
                return gsup

            def qt_slice(lx, j, cx):
                if lx == 0:
                    if j == 0:
                        return qt00_sb[:, :, cx * 128 : (cx + 1) * 128]
                    return qt0r_sb[
                        :, 2 * (j - 1) : 2 * j, cx * 128 : (cx + 1) * 128
                    ]
                k0 = (lx - 1) * KC + 2 * j
                return qtr_sb[:, k0 : k0 + 2, cx * 128 : (cx + 1) * 128]

            def stage_b(lx, m0, cols, off, init_w, gsup):
                j0 = 2 * lx
                if m0 == MS - 256:
                    # final 256-col tile of this layer: matmuls, then reduce
                    # the PSUM directly and fold into the (already partially
                    # reduced) lmax -- keeps the post-last-matmul tail to a
                    # PSUM reduce + tiny combine + tiny DMA.
                    pm = pmpool.tile([128, 2, SUPER], F32, name="pm")
                    for cx in range(2):
                        for j in range(KP):
                            nc.tensor.matmul(
                                pm[:, cx, :256],
                                qt_slice(lx, j, cx),
                                gsup[:, 2 * j : 2 * j + 2, :],
                                start=(j == 0),
                                stop=(j == KP - 1),
                                perf_mode=DR,
                            )
                    tmp = tmppool.tile([128, 2], F32, name="tmax")
                    nc.vector.reduce_max(
                        tmp[:], pm[:, :, :256], axis=mybir.AxisListType.X
                    )
                    # combine into the shared output staging tile; ONE DMA
                    # ships all layers at the end
                    nc.vector.tensor_tensor(
                        out=out_all[:, j0 : j0 + 2],
                        in0=lmax_l[lx][:],
                        in1=tmp[:],
                        op=ALU.max,
                    )
                    return
                # consume the gallery tile in 512-col supers
                for s0 in range(0, cols, SUPER):
                    msz = min(SUPER, cols - s0)
                    pm = pmpool.tile([128, 2, SUPER], F32, name="pm")
                    for cx in range(2):
                        for j in range(KP):
                            nc.tensor.matmul(
                                pm[:, cx, :msz],
                                qt_slice(lx, j, cx),
                                gsup[:, 2 * j : 2 * j + 2, s0 : s0 + msz],
                                start=(j == 0),
                                stop=(j == KP - 1),
                                perf_mode=DR,
                            )
                    if m0 + s0 >= DIRECT_FROM:
                        # endgame super: reduce PSUM directly, fold into the
                        # (already partially reduced) lmax
                        tmp = tmppool.tile([128, 2], F32, name="tmax")
                        nc.vector.reduce_max(
                            tmp[:], pm[:, :, :msz], axis=mybir.AxisListType.X
                        )
                        nc.vector.tensor_tensor(
                            out=lmax_l[lx][:],
                            in0=lmax_l[lx][:],
                            in1=tmp[:],
                            op=ALU.max,
                        )
                        continue
                    # running elementwise max into bf16 runmax (DVE); first
                    # touch of a column range copies instead of maxing
                    w = min(init_w, msz)
                    if w > 0:
                        rm = runmax[:, j0 : j0 + 2, :w]
                        nc.vector.tensor_tensor(
                            out=rm, in0=rm, in1=pm[:, :, :w], op=ALU.max
                        )
                    if msz > w:
                        nc.vector.tensor_copy(
                            out=runmax[:, j0 : j0 + 2, w:msz],
                            in_=pm[:, :, w:msz],
                        )
                    init_w = max(init_w, msz)
                if m0 + cols == DIRECT_FROM:
                    # all runmax supers of this layer done: partial reduce
                    # NOW (overlaps the endgame tiles' matmuls instead of
                    # sitting on the serial tail)
                    nc.vector.reduce_max(
                        lmax_l[lx][:],
                        runmax[:, j0 : j0 + 2, :],
                        axis=mybir.AxisListType.X,
                    )

            # init_w = how many runmax columns are valid before this tile
            # (running max of prior super widths within the layer)
            work = []
            _iw = {lx: 0 for lx in range(L)}
            for lx, m0, cols, off in GTILES:
                work.append((lx, m0, cols, off, _iw[lx]))
                _iw[lx] = max(_iw[lx], min(SUPER, cols))
            pending = []
            for sidx, (lx, m0, cols, off, iw) in enumerate(work):
                pending.append((lx, m0, cols, off, iw, stage_a(lx, m0, cols, off, sidx)))
                if sidx == 0:
                    # layer 0's query slices, on the other HWDGE ring so
                    # they drain in parallel with the first gallery tile
                    nc.scalar.dma_start(
                        qt00_sb[:],
                        qt_ext.ap()[:, : 2 * N].rearrange(
                            "p (k n) -> p k n", k=2
                        ),
                    )
                    nc.scalar.dma_start(
                        qt0r_sb[:],
                        qt_ext.ap()[:, 2 * N : KC * N].rearrange(
                            "p (k n) -> p k n", k=KC - 2
                        ),
                    )
                elif sidx == 5:
                    # remaining layers' queries: deferred so they don't
                    # steal early DMA bandwidth from the first gallery tiles
                    nc.scalar.dma_start(
                        qtr_sb[:],
                        qt_ext.ap()[:, KC * N :].rearrange(
                            "p (k n) -> p k n", k=(L - 1) * KC
                        ),
                    )
                if len(pending) > SKEW:
                    stage_b(*pending.pop(0))
            while pending:
                stage_b(*pending.pop(0))
            # single result DMA on the low-latency HWDGE scalar ring
            nc.scalar.dma_start(lmax_ext.ap(), out_all[:])

    nc.compile()
    return nc


_NC_CACHE = None


def _get_nc():
    global _NC_CACHE
    if _NC_CACHE is None:
        _NC_CACHE = build()
    return _NC_CACHE


def _prep_shard(g, c):
    # [L, MS, D] f32 slice -> l2-normalize * SCALE -> fp8 -> per-tile
    # partition-flat blob [128, G_FLAT]
    sl = np.asarray(g[:, c * MS : (c + 1) * MS, :], dtype=np.float32)
    n = np.sqrt(np.einsum("lmd,lmd->lm", sl, sl))
    sl = sl * (SCALE / np.maximum(n, 1e-8))[:, :, None]
    gt = sl.transpose(0, 2, 1).astype(NP_IN)  # [L, D, MS]
    blob = np.empty((128, G_FLAT), dtype=NP_IN)
    for lx, m0, cols, off in GTILES:
        t = gt[lx, :, m0 : m0 + cols].reshape(KC, 128, cols)
        blob[:, off : off + KC * cols] = t.transpose(1, 0, 2).reshape(
            128, KC * cols
        )
    return blob


def _prep_inputs(test_patch_tokens, normal_patch_tokens):
    q = np.asarray(test_patch_tokens, dtype=np.float32)
    g = np.asarray(normal_patch_tokens, dtype=np.float32)
    qn = np.sqrt(np.einsum("lnd,lnd->ln", q, q))
    q = q * (SCALE / np.maximum(qn, 1e-8))[:, :, None]
    qt = q.transpose(0, 2, 1).astype(NP_IN)  # [L, D, N]
    qt_flat = np.ascontiguousarray(
        qt.reshape(L, KC, 128, N).transpose(2, 0, 1, 3).reshape(128, L * KC * N)
    )
    with ThreadPoolExecutor(NCORES) as ex:
        shards = list(ex.map(lambda c: _prep_shard(g, c), range(NCORES)))
    return [{"g_t": shards[c], "q_t": qt_flat} for c in range(NCORES)]


def kernel(test_patch_tokens: np.ndarray, normal_patch_tokens: np.ndarray):
    in_maps = _prep_inputs(test_patch_tokens, normal_patch_tokens)
    nc = _get_nc()
    results = run_bass_kernel_spmd(nc, in_maps, core_ids=list(range(NCORES))).results
    # combine per-shard partials: global max over cores, /SCALE^2, mean over
    # layers, 1-x (tiny: 8*[128,8] values)
    lmax = np.max(
        np.stack([results[c]["out_lmax"] for c in range(NCORES)]), axis=0
    )  # [128, 2L]: column j = layer*2 + chunk
    test_sim = (lmax / (SCALE * SCALE)).reshape(128, L, 2).mean(axis=1)
    out = 1.0 - test_sim.T.reshape(N)  # n = chunk*128 + p
    return out.astype(np.float32).reshape(1, 1, 16, 16)


# revision 35
# speedup vs baseline: 1.0084x; 1.0003x over previous
"""Distributed Trainium2 kernel for AnomalyMoE k-NN retrieval.

reference:  q = l2norm(test[L,N,D]); g = l2norm(normal[L,M,D])
            sim[l,n,m] = q . g ; out = (1 - mean_l max_m sim).reshape(1,1,16,16)

Strategy (8 NeuronCores):
- Shard gallery along M (6400 rows/core).  ALL normalization happens on the
  host: q and g are l2-normalized in f32, scaled by 16 (keeps fp8e4m3
  components out of the subnormal range), cast to fp8.
- The gallery shard is shipped as ONE partition-flat blob [128, 204800]
  laid out per-DMA-tile: each tile (layer, m-range) occupies a contiguous
  column range holding [128p, KC, cols] row-major.  Every gallery DMA is
  then 128 contiguous 2-16KB descriptors -> near-peak HBM bandwidth and
  ~0.25us issue cost (vs 1024 small descriptors at ~300GB/s).  Queries ship
  the same way ([128, L*KC*N] flat).
- Per core the device does ONLY:  dot[n,m] on TensorE fp8 DoubleRow
  pair-matmuls (contraction 256/instruction, FD=512 into f32 PSUM), a
  running elementwise max on DVE (bf16 runmax), one partial 512->1 reduce
  per layer (emitted early, off the critical tail), and for each layer's
  final 256-col tile a direct PSUM reduce + combine.  Warm-up matmuls on
  junk data run during the initial DMA window so HAM is at K=8/8 when the
  real stream starts.
- Gallery DMAs alternate between the two HWDGE rings (sync + scalar);
  tiny per-layer [128,2] result DMAs go out on the GpSimd SWDGE ring.
- Host: cross-core max, /scale^2, mean over layers, 1-x.
"""

import os
import sys
from concurrent.futures import ThreadPoolExecutor

sys.path.insert(0, "/opt/trn_rl_repo")

import numpy as np
import ml_dtypes

import concourse.bacc as bacc
import concourse.mybir as mybir
import concourse.tile as tile
from concourse.bass_utils import run_bass_kernel_spmd

F32 = mybir.dt.float32
BF16 = mybir.dt.bfloat16
ALU = mybir.AluOpType
DR = mybir.MatmulPerfMode.DoubleRow
DT_IN = mybir.dt.float8e4
NP_IN = ml_dtypes.float8_e4m3fn

NCORES = 8
L = 4
D = 1024
N = 256
M_FULL = 51200
MS = M_FULL // NCORES  # 6400 per core
KC = D // 128  # 8 contraction chunks of 128
KP = KC // 2  # 4 DoubleRow pairs
SUPER = 512  # matmul FD (one f32 PSUM bank)
SCALE = 16.0  # host pre-scale on normalized q and g (sims come out x256)
SKEW = int(os.environ.get("KERNEL_SKEW", "1"))  # software-pipeline depth
BUFS_G = int(os.environ.get("KERNEL_BUFS_G", "6"))
BUFS_PM = int(os.environ.get("KERNEL_BUFS_PM", "3"))
N_WARM = int(os.environ.get("KERNEL_WARM", "6"))
KERNEL_TAG = os.environ.get("KERNEL_TAG", "")
NEG = -3.0e38

# per-layer DMA tile column counts.  PE consumes ~263KB/us while the DMA
# supplies ~310KB/us, so early tiles must be SMALL (completion cadence at
# or below the PE cadence) and can ramp up as prefetch slack accumulates.
# Each layer ends with a 256 tile handled by a direct PSUM reduce.
TILES = {
    0: [256, 256, 256, 256, 512, 512, 1024, 1024, 1024, 1024, 256],
    1: [512, 512, 1024, 1024, 1024, 1024, 1024, 256],
    2: [512, 512, 1024, 1024, 1024, 1024, 1024, 256],
    3: [512, 512, 1024, 1024, 1024, 1024, 1024, 256],
}
# supers at columns >= DIRECT_FROM skip the runmax and reduce their PSUM
# directly (tiny tmp + combine): the per-layer partial reduce then runs
# four supers early and each endgame reduce starts as soon as its PSUM is
# ready, shrinking the serial DVE chain after the last matmul
DIRECT_FROM = MS - 256 - 2048  # 4096

# work list: (layer, m0, cols, flat column offset in the gallery blob)
GTILES = []
_off = 0
for _lx in range(L):
    _m0 = 0
    for _c in TILES[_lx]:
        GTILES.append((_lx, _m0, _c, _off))
        _m0 += _c
        _off += KC * _c
    assert _m0 == MS
G_FLAT = _off  # 204800


def build():
    nc = bacc.Bacc("TRN2", target_bir_lowering=False, debug=False, num_devices=NCORES)
    g_ext = nc.dram_tensor("g_t", [128, G_FLAT], DT_IN, kind="ExternalInput")
    qt_ext = nc.dram_tensor("q_t", [128, L * KC * N], DT_IN, kind="ExternalInput")
    lmax_ext = nc.dram_tensor("out_lmax", [128, 2 * L], F32, kind="ExternalOutput")

    with tile.TileContext(nc) as tc:
        with (
            tc.tile_pool(name="persist", bufs=1) as pp,
            tc.tile_pool(name="gsup", bufs=BUFS_G) as gpool,
            tc.tile_pool(name="tmpp", bufs=2) as tmppool,
            tc.tile_pool(name="pmp", bufs=BUFS_PM, space="PSUM") as pmpool,
        ):
            # ---- persistent tiles ----
            # qt in TWO tiles (layer 0 / layers 1-3) and lmax per-layer:
            # separate tiles kill the false WAR/RAW deps that a shared tile
            # creates between the layer-0 LDWEIGHTS and the qt-rest DMA, and
            # between a layer's partial reduce and the previous layer's
            # result DMA.
            # qt00 = just layer 0's first k-pair (64KB): the very first
            # LDWEIGHTS waits only on this tiny DMA, not the full qt0 load
            qt00_sb = pp.tile([128, 2, N], DT_IN, name="qt00_sb")
            qt0r_sb = pp.tile([128, KC - 2, N], DT_IN, name="qt0r_sb")
            qtr_sb = pp.tile([128, (L - 1) * KC, N], DT_IN, name="qtr_sb")
            lmax_l = [
                pp.tile([128, 2], F32, name=f"lmax_l{_l}") for _l in range(L)
            ]
            out_all = pp.tile([128, 2 * L], F32, name="out_all")
            runmax = pp.tile([128, 2 * L, SUPER], BF16, name="runmax")
            # runmax columns are initialized by copy the first time each
            # column range is touched (init_w); no memset needed.
            if KERNEL_TAG:
                # cache-buster: changes the BIR so stale NEFF caches miss
                tag_sb = pp.tile([128, 1], F32, name=f"tag_{KERNEL_TAG}")
                nc.gpsimd.memset(tag_sb[:], 1.0)

            # ---- PE warm-up: junk DoubleRow matmuls during the initial DMA
            # window keep the HAM activity monitor busy so the real stream
            # starts at K=8/8 (2.4 GHz) instead of cold (1.2 GHz).
            if N_WARM:
                wq = pp.tile([128, 2, 128], DT_IN, name="warm_q")
                wx = pp.tile([128, 2, SUPER], DT_IN, name="warm_x")
                nc.gpsimd.memset(wq[:], 1.0)
                nc.gpsimd.memset(wx[:], 1.0)
                pmw = pmpool.tile([128, 2, SUPER], F32, name="pm")
                for _ in range(N_WARM):
                    nc.tensor.matmul(
                        pmw[:, 0, :], wq[:], wx[:], start=True, stop=True,
                        perf_mode=DR,
                    )

            def stage_a(lx, m0, cols, off, sidx):
                gsup = gpool.tile([128, KC, cols], DT_IN, name="gsup")
                eng = nc.sync if sidx % 2 == 0 else nc.scalar
                eng.dma_start(
                    gsup[:],
                    g_ext.ap()[:, off : off + KC * cols].rearrange(
                        "p (k m) -> p k m", k=KC
                    ),
                )
                return gsup

            def qt_slice(lx, j, cx):
                if lx == 0:
                    if j == 0:
                        return qt00_sb[:, :, cx * 128 : (cx + 1) * 128]
                    return qt0r_sb[
                        :, 2 * (j - 1) : 2 * j, cx * 128 : (cx + 1) * 128
                    ]
                k0 = (lx - 1) * KC + 2 * j
                return qtr_sb[:, k0 : k0 + 2, cx * 128 : (cx + 1) * 128]

            def stage_b(lx, m0, cols, off, init_w, gsup):
                j0 = 2 * lx
                if m0 == MS - 256:
                    # final 256-col tile of this layer: matmuls, then reduce
                    # the PSUM directly and fold into the (already partially
                    # reduced) lmax -- keeps the post-last-matmul tail to a
                    # PSUM reduce + tiny combine + tiny DMA.
                    pm = pmpool.tile([128, 2, SUPER], F32, name="pm")
                    for cx in range(2):
                        for j in range(KP):
                            nc.tensor.matmul(
                                pm[:, cx, :256],
                                qt_slice(lx, j, cx),
                                gsup[:, 2 * j : 2 * j + 2, :],
                                start=(j == 0),
                                stop=(j == KP - 1),
                                perf_mode=DR,
                            )
                    tmp = tmppool.tile([128, 2], F32, name="tmax")
                    nc.vector.reduce_max(
                        tmp[:], pm[:, :, :256], axis=mybir.AxisListType.X
                    )
                    # combine into the shared output staging tile; ONE DMA
                    # ships all layers at the end
                    nc.vector.tensor_tensor(
                        out=out_all[:, j0 : j0 + 2],
                        in0=lmax_l[lx][:],
                        in1=tmp[:],
                        op=ALU.max,
                    )
                    return
                # consume the gallery tile in 512-col supers
                for s0 in range(0, cols, SUPER):
                    msz = min(SUPER, cols - s0)
                    pm = pmpool.tile([128, 2, SUPER], F32, name="pm")
                    for cx in range(2):
                        for j in range(KP):
                            nc.tensor.matmul(
                                pm[:, cx, :msz],
                                qt_slice(lx, j, cx),
                                gsup[:, 2 * j : 2 * j + 2, s0 : s0 + msz],
                                start=(j == 0),
                                stop=(j == KP - 1),
                                perf_mode=DR,
                            )
                    if m0 + s0 >= DIRECT_FROM:
                        # endgame super: reduce PSUM directly, fold into the
                        # (already partially reduced) lmax
                        tmp = tmppool.tile([128, 2], F32, name="tmax")
                        nc.vector.reduce_max(
                            tmp[:], pm[:, :, :msz], axis=mybir.AxisListType.X
                        )
                        nc.vector.tensor_tensor(
                            out=lmax_l[lx][:],
                            in0=lmax_l[lx][:],
                            in1=tmp[:],
                            op=ALU.max,
                        )
                        continue
                    # running elementwise max into bf16 runmax (DVE); first
                    # touch of a column range copies instead of maxing
                    w = min(init_w, msz)
                    if w > 0:
                        rm = runmax[:, j0 : j0 + 2, :w]
                        nc.vector.tensor_tensor(
                            out=rm, in0=rm, in1=pm[:, :, :w], op=ALU.max
                        )
                    if msz > w:
                        nc.vector.tensor_copy(
                            out=runmax[:, j0 : j0 + 2, w:msz],
                            in_=pm[:, :, w:msz],
                        )
                    init_w = max(init_w, msz)
                if m0 + cols == DIRECT_FROM:
                    # all runmax supers of this layer done: partial reduce
                    # NOW (overlaps the endgame tiles' matmuls instead of
                    # sitting on the serial tail)
                    nc.vector.reduce_max(
                        lmax_l[lx][:],
                        runmax[:, j0 : j0 + 2, :],
                        axis=mybir.AxisListType.X,
                    )

            # init_w = how many runmax columns are valid before this tile
            # (running max of prior super widths within the layer)
            work = []
            _iw = {lx: 0 for lx in range(L)}
            for lx, m0, cols, off in GTILES:
                work.append((lx, m0, cols, off, _iw[lx]))
                _iw[lx] = max(_iw[lx], min(SUPER, cols))
            pending = []
            for sidx, (lx, m0, cols, off, iw) in enumerate(work):
                pending.append((lx, m0, cols, off, iw, stage_a(lx, m0, cols, off, sidx)))
                if sidx == 0:
                    # layer 0's query slices, on the other HWDGE ring so
                    # they drain in parallel with the first gallery tile
                    nc.scalar.dma_start(
                        qt00_sb[:],
                        qt_ext.ap()[:, : 2 * N].rearrange(
                            "p (k n) -> p k n", k=2
                        ),
                    )
                    nc.scalar.dma_start(
                        qt0r_sb[:],
                        qt_ext.ap()[:, 2 * N : KC * N].rearrange(
                            "p (k n) -> p k n", k=KC - 2
                        ),
                    )
                elif sidx == 5:
                    # remaining layers' queries: deferred so they don't
                    # steal early DMA bandwidth from the first gallery tiles
                    nc.scalar.dma_start(
                        qtr_sb[:],
                        qt_ext.ap()[:, KC * N :].rearrange(
                            "p (k n) -> p k n", k=(L - 1) * KC
                        ),
                    )
                if len(pending) > SKEW:
                    stage_b(*pending.pop(0))
            while pending:
                stage_b(*pending.pop(0))
            # single result DMA on the low-latency HWDGE scalar ring
            nc.scalar.dma_start(lmax_ext.ap(), out_all[:])

    nc.compile()
    return nc


_NC_CACHE = None


def _get_nc():
    global _NC_CACHE
    if _NC_CACHE is None:
        _NC_CACHE = build()
    return _NC_CACHE


def _prep_shard(g, c):
    # [L, MS, D] f32 slice -> l2-normalize * SCALE -> fp8 -> per-tile
    # partition-flat blob [128, G_FLAT]
    sl = np.asarray(g[:, c * MS : (c + 1) * MS, :], dtype=np.float32)
    n = np.sqrt(np.einsum("lmd,lmd->lm", sl, sl))
    sl = sl * (SCALE / np.maximum(n, 1e-8))[:, :, None]
    gt = sl.transpose(0, 2, 1).astype(NP_IN)  # [L, D, MS]
    blob = np.empty((128, G_FLAT), dtype=NP_IN)
    for lx, m0, cols, off in GTILES:
        t = gt[lx, :, m0 : m0 + cols].reshape(KC, 128, cols)
        blob[:, off : off + KC * cols] = t.transpose(1, 0, 2).reshape(
            128, KC * cols
        )
    return blob


def _prep_inputs(test_patch_tokens, normal_patch_tokens):
    q = np.asarray(test_patch_tokens, dtype=np.float32)
    g = np.asarray(normal_patch_tokens, dtype=np.float32)
    qn = np.sqrt(np.einsum("lnd,lnd->ln", q, q))
    q = q * (SCALE / np.maximum(qn, 1e-8))[:, :, None]
    qt = q.transpose(0, 2, 1).astype(NP_IN)  # [L, D, N]
    qt_flat = np.ascontiguousarray(
        qt.reshape(L, KC, 128, N).transpose(2, 0, 1, 3).reshape(128, L * KC * N)
    )
    with ThreadPoolExecutor(NCORES) as ex:
        shards = list(ex.map(lambda c: _prep_shard(g, c), range(NCORES)))
    return [{"g_t": shards[c], "q_t": qt_flat} for c in range(NCORES)]


def kernel(test_patch_tokens: np.ndarray, normal_patch_tokens: np.ndarray):
    in_maps = _prep_inputs(test_patch_tokens, normal_patch_tokens)
    nc = _get_nc()
    results = run_bass_kernel_spmd(nc, in_maps, core_ids=list(range(NCORES))).results
    # combine per-shard partials: global max over cores, /SCALE^2, mean over
    # layers, 1-x (tiny: 8*[128,8] values)
    lmax = np.max(
        np.stack([results[c]["out_lmax"] for c in range(NCORES)]), axis=0
    )  # [128, 2L]: column j = layer*2 + chunk
    test_sim = (lmax / (SCALE * SCALE)).reshape(128, L, 2).mean(axis=1)
    out = 1.0 - test_sim.T.reshape(N)  # n = chunk*128 + p
    return out.astype(np.float32).reshape(1, 1, 16, 16)
